# revision 1
# baseline (speedup 1.0000x reference)
"""Multi-head attention kernel for Trainium2, 8 NeuronCores.

Problem: B=2, S=4096, D=512, H=8 heads (dk=64), explicit S x S masked softmax.

Sharding: batch (2) x query-row-blocks (4) -> 8 cores. Each core computes all 8
heads for 1024 query rows of one batch element. K/V projections are computed
per-core for the full sequence (duplicated across the 4 cores of a batch).

Per-core layout choices:
  - scores computed transposed ([keys, q]) so the PV matmul consumes them
    directly (no on-chip transposes anywhere).
  - mask is host-transposed to [S, QR] and DMA-cast int32->bf16 on load.
  - softmax sums come from a ones-column appended to V (M=65 PV matmul);
    the reciprocal row is broadcast across partitions via a DRAM bounce
    (stride-0 partition DMA) and the normalize-multiply of each head is
    deferred into the next head's pipeline so unit boundaries never stall.
  - all matmul operands bf16 (DMA-cast on load), fp32 accumulation in PSUM.
  - emission is a flat software pipeline over (head, group) with score
    matmuls issued two blocks ahead; output-projection chunks are spread
    across subsequent heads. ScalarE (exp) runs at ~97-100%% occupancy.
"""

import numpy as np

B, S, D, H = 2, 4096, 512, 8
DK = D // H            # 64
NCORES = 8
RG = 4                 # row groups per batch
QR = S // RG           # 1024 query rows per core
QT = 512               # query tile
NQT = QR // QT         # 2
KBS = 128              # key block size
KB = S // KBS          # 32 key blocks
G = 3                  # key blocks per ACT exp group (3 PSUM banks, FD=1536)

_BUILT = None


def _build():
    import concourse.bacc as bacc
    import concourse.mybir as mybir
    import concourse.tile as tile
    from concourse.bass_interp import get_hw_module

    F32 = mybir.dt.float32
    BF16 = mybir.dt.bfloat16
    I32 = mybir.dt.int32
    EXP = mybir.ActivationFunctionType.Exp
    MULT = mybir.AluOpType.mult

    nc = bacc.Bacc("TRN2", target_bir_lowering=False, debug=False,
                   enable_asserts=False, num_devices=NCORES)

    qT = nc.dram_tensor("qT", [D, QR], F32, kind="ExternalInput")
    kT = nc.dram_tensor("kT", [D, S], F32, kind="ExternalInput")
    vT = nc.dram_tensor("vT", [D, S], F32, kind="ExternalInput")
    maskT = nc.dram_tensor("maskT", [S, QR], I32, kind="ExternalInput")
    wq = nc.dram_tensor("wq", [D, D], F32, kind="ExternalInput")
    wk = nc.dram_tensor("wk", [D, D], F32, kind="ExternalInput")
    wv = nc.dram_tensor("wv", [D, D], F32, kind="ExternalInput")
    wo = nc.dram_tensor("wo", [D, D], F32, kind="ExternalInput")
    out = nc.dram_tensor("out", [QR, D], F32, kind="ExternalOutput")
    # DRAM bounce buffer for broadcasting softmax reciprocals across partitions
    rcd = nc.dram_tensor("rcd", [NQT * H, 512], F32, kind="Internal")

    with tile.TileContext(nc) as tc:
        with tc.tile_pool(name="persist", bufs=1) as persist, \
             tc.tile_pool(name="maskp", bufs=1) as maskp:

            # persistent tiles
            KT = persist.tile([128, 4, S], BF16)      # K^T, 4 d_out chunks
            QTt = persist.tile([128, 4, QR], BF16)    # Q^T
            VA = persist.tile([128, KB, H * 65], BF16)  # V + ones col per head
            maskA = maskp.tile([128, KB, QT], BF16)
            ones_t = persist.tile([128, 64], F32)
            nc.vector.memset(ones_t, 1.0)

            # ones column of VA (head-interleaved: col h*65+64)
            va_ones = VA.rearrange("p kb (h x) -> p kb h x", x=65)[:, :, :, 64:65]
            nc.gpsimd.memset(va_ones, 1.0)

            mask_src = maskT[:, :].rearrange("(kb p) q -> p kb q", p=128)

            # ---------------- projections ----------------
            # Order: V first, then K(dc0) + Q(first tile), then two "warmup"
            # attention heads with small exp groups run during the remaining
            # projection work (ACT is otherwise idle there), then the rest.
            HS = S // 2  # 2048
            NW = 0       # warmup units
            units = [(qt, h) for qt in range(NQT) for h in range(H)]
            xts = {}

            with tc.tile_pool(name="pxt", bufs=2) as pxt, \
                 tc.tile_pool(name="pwrk", bufs=3) as pwrk:

                import concourse.bass as bass

                def emit_norm1(ui, pv):
                    # reciprocal of the sums row, bounced through DRAM to
                    # broadcast it across partitions 0-63 (SBUF APs cannot
                    # have a zero partition stride; DRAM APs can)
                    rc = pwrk.tile([128, QT], F32, tag="rc", name=f"rc{ui}")
                    nc.vector.reciprocal(rc[64:65, :], pv[64:65, :])
                    nc.sync.dma_start(out=rcd[ui:ui + 1, :], in_=rc[64:65, :])
                    bcs = pwrk.tile([64, QT], F32, tag="bcs", name=f"bcs{ui}")
                    src = rcd[ui:ui + 1, :]
                    bsrc = bass.AP(tensor=src.tensor, offset=src.offset,
                                   ap=[[0, 64]] + [list(a) for a in src.ap[1:]])
                    nc.sync.dma_start(out=bcs, in_=bsrc)
                    return bcs

                def emit_norm2(qt, h, pv, bcs):
                    nc.vector.tensor_tensor(xts[qt][:, h, :], pv[0:64, :],
                                            bcs, op=MULT)

                with tc.tile_pool(name="pin", bufs=1) as pin, \
                     tc.tile_pool(name="pint", bufs=2) as pint, \
                     tc.tile_pool(name="pexw", bufs=3) as pexw, \
                     tc.tile_pool(name="pps", bufs=4, space="PSUM") as pps, \
                     tc.tile_pool(name="pscw", bufs=2, space="PSUM") as pscw, \
                     tc.tile_pool(name="ppvw", bufs=1, space="PSUM") as ppvw:

                    pscw._bctag = "scw"
                    wk_bf = pin.tile([128, 4, D], BF16, tag="wk")
                    wq_bf = pin.tile([128, 4, D], BF16, tag="wq")
                    wv_bf = pin.tile([128, 4, D], BF16, tag="wv")
                    qtin = pin.tile([128, 4, QR], BF16, tag="qtin")
                    kT_src = kT[:, :].rearrange("(c p) s -> p c s", p=128)
                    vT_src = vT[:, :].rearrange("(c p) s -> p c s", p=128)

                    nc.gpsimd.dma_start(
                        out=wk_bf,
                        in_=wk[:, :].rearrange("(c p) d -> p c d", p=128))

                    # K^T projection, half-slab major
                    for hf in range(2):
                        ktin = pint.tile([128, 4, HS], BF16, tag="tin",
                                         name=f"ktin{hf}")
                        for qh in range(2):
                            a = hf * HS + qh * (HS // 2)
                            nc.gpsimd.dma_start(
                                out=ktin[:, :, qh * (HS // 2):
                                         (qh + 1) * (HS // 2)],
                                in_=kT_src[:, :, a:a + HS // 2])
                        if hf == 1:
                            nc.gpsimd.dma_start(
                                out=wq_bf,
                                in_=wq[:, :].rearrange("(c p) d -> p c d",
                                                       p=128))
                            nc.gpsimd.dma_start(
                                out=wv_bf,
                                in_=wv[:, :].rearrange("(c p) d -> p c d",
                                                       p=128))
                            nc.gpsimd.dma_start(
                                out=qtin,
                                in_=qT[:, :].rearrange("(c p) r -> p c r",
                                                       p=128))
                        for dc in range(4):
                            for ns in range(HS // 512):
                                s0 = hf * HS + ns * 512
                                pt = pps.tile([128, 512], F32, tag="pt",
                                              name=f"ptk{dc}_{hf}_{ns}")
                                for di in range(4):
                                    nc.tensor.matmul(
                                        pt,
                                        wk_bf[:, di, dc * 128:(dc + 1) * 128],
                                        ktin[:, di,
                                             ns * 512:(ns + 1) * 512],
                                        start=(di == 0), stop=(di == 3))
                                nc.vector.tensor_copy(
                                    KT[:, dc, s0:s0 + 512], pt)

                    # Q^T projection
                    for dc in range(4):
                        for ns in range(QR // 512):
                            pt = pps.tile([128, 512], F32, tag="pt",
                                          name=f"ptq{dc}_{ns}")
                            for di in range(4):
                                nc.tensor.matmul(
                                    pt,
                                    wq_bf[:, di, dc * 128:(dc + 1) * 128],
                                    qtin[:, di, ns * 512:(ns + 1) * 512],
                                    start=(di == 0), stop=(di == 3))
                            nc.vector.tensor_copy(
                                QTt[:, dc, ns * 512:(ns + 1) * 512], pt)

                    nc.gpsimd.dma_start(out=maskA[:, 0:8, :],
                                        in_=mask_src[:, 0:8, 0:QT])

                    # V projection, half-slab major, scattered into VA
                    for hf in range(2):
                        vtin = pint.tile([128, 4, HS], BF16, tag="tin",
                                         name=f"vtin{hf}")
                        for qh in range(2):
                            a = hf * HS + qh * (HS // 2)
                            nc.gpsimd.dma_start(
                                out=vtin[:, :, qh * (HS // 2):
                                         (qh + 1) * (HS // 2)],
                                in_=vT_src[:, :, a:a + HS // 2])
                        for si in range(HS // 128):
                            sc_i = hf * (HS // 128) + si
                            pt = pps.tile([128, 512], F32, tag="pt",
                                          name=f"ptv{sc_i}")
                            for di in range(4):
                                nc.tensor.matmul(
                                    pt,
                                    vtin[:, di, si * 128:(si + 1) * 128],
                                    wv_bf[:, di, :],
                                    start=(di == 0), stop=(di == 3))
                            dst = VA[:, sc_i, :].rearrange(
                                "p (h x) -> p h x", x=65)[:, :, 0:64]
                            nc.vector.tensor_copy(
                                dst, pt.rearrange("p (h x) -> p h x", x=64))

                    nc.gpsimd.dma_start(out=maskA[:, 8:16, :],
                                        in_=mask_src[:, 8:16, 0:QT])
                    nc.gpsimd.dma_start(out=maskA[:, 16:24, :],
                                        in_=mask_src[:, 16:24, 0:QT])
                    nc.gpsimd.dma_start(out=maskA[:, 24:32, :],
                                        in_=mask_src[:, 24:32, 0:QT])

                # ---------------- main attention ----------------
                with tc.tile_pool(name="late", bufs=1) as late, \
                     tc.tile_pool(name="pex", bufs=4) as pex, \
                     tc.tile_pool(name="psc", bufs=2, space="PSUM") as psc, \
                     tc.tile_pool(name="ppv", bufs=2, space="PSUM") as ppv:

                    ppv._bctag = "pvb"
                    WO64 = late.tile([64, H, D], BF16)
                    nc.gpsimd.dma_start(
                        out=WO64,
                        in_=wo[:, :].rearrange("(h p) d -> p h d", p=64))
                    maskB = late.tile([128, KB, QT], BF16)
                    for mc in range(4):
                        nc.gpsimd.dma_start(
                            out=maskB[:, mc * 8:(mc + 1) * 8, :],
                            in_=mask_src[:, mc * 8:(mc + 1) * 8, QT:QR])

                    groups = []
                    kb0 = 0
                    while kb0 < KB:
                        groups.append((kb0, min(G, KB - kb0)))
                        kb0 += G

                    def emit_scores(qt, h, kb0, gn):
                        pb = (h % 2) * 64
                        hc = h // 2
                        sc = psc.tile([128, G, QT], F32, tag="sc")
                        for i in range(gn):
                            kb = kb0 + i
                            nc.tensor.matmul(
                                sc[:, i, :],
                                KT[pb:pb + 64, hc, kb * 128:(kb + 1) * 128],
                                QTt[pb:pb + 64, hc, qt * QT:(qt + 1) * QT],
                                start=True, stop=True)
                        return sc

                    pending_wo = []

                    def emit_wo(qt, qc):
                        xt = xts[qt]
                        po = ppv.tile([128, D], F32, tag="pvb",
                                      name=f"po{qt}_{qc}")
                        for hh in range(H):
                            nc.tensor.matmul(
                                po,
                                xt[:, hh, qc * 128:(qc + 1) * 128],
                                WO64[:, hh, :],
                                start=(hh == 0), stop=(hh == H - 1))
                        outt = late.tile([128, D], F32, tag="outt", bufs=3)
                        nc.vector.tensor_copy(outt, po)
                        nc.sync.dma_start(
                            out=out[qt * QT + qc * 128:
                                    qt * QT + (qc + 1) * 128, :],
                            in_=outt)

                    # flat software pipeline over (unit, group); scores are
                    # emitted two blocks ahead so ACT never waits on the PE
                    flat = []
                    for ui in range(NW, len(units)):
                        qt, h = units[ui]
                        for gi, (kb0, gn) in enumerate(groups):
                            flat.append((ui, qt, h, gi, kb0, gn))

                    sc_tiles = {}

                    def emit_sc(idx):
                        _, qt, h, _, kb0, gn = flat[idx]
                        sc_tiles[idx] = emit_scores(qt, h, kb0, gn)

                    emit_sc(0)
                    emit_sc(1)
                    pv = None
                    pending_norm = None
                    wo_inflight = None
                    for idx, (ui, qt, h, gi, kb0, gn) in enumerate(flat):
                        if h == 0 and gi == 0 and qt not in xts:
                            xts[qt] = pxt.tile([64, H, QT], BF16, tag="xt",
                                               name=f"xt{qt}")
                        xt = xts[qt]
                        if gi == 0:
                            pv = ppv.tile([128, QT], F32, tag="pvb",
                                          name=f"pv{ui}")
                        if idx + 2 < len(flat):
                            emit_sc(idx + 2)
                        sc = sc_tiles.pop(idx)
                        ex = pex.tile([128, G, QT], BF16, tag="ex")
                        nc.scalar.activation(ex[:, 0:gn, :], sc[:, 0:gn, :],
                                             EXP, scale=0.125)
                        mk = pex.tile([128, G, QT], BF16, tag="mk")
                        nc.vector.tensor_tensor(
                            mk[:, 0:gn, :], ex[:, 0:gn, :],
                            (maskA if qt == 0 else maskB)[:, kb0:kb0 + gn, :],
                            op=MULT)
                        for i in range(gn):
                            kb = kb0 + i
                            nc.tensor.matmul(
                                pv[0:65, :],
                                VA[:, kb, h * 65:(h + 1) * 65],
                                mk[:, i, :],
                                start=(kb == 0), stop=(kb == KB - 1))
                        if wo_inflight is not None and gi == 7:
                            qtw, qcw, po = wo_inflight
                            wo_inflight = None
                            for hh in range(4, H):
                                nc.tensor.matmul(
                                    po,
                                    xts[qtw][:, hh, qcw * 128:(qcw + 1) * 128],
                                    WO64[:, hh, :],
                                    start=False, stop=(hh == H - 1))
                            outt = late.tile([128, D], F32, tag="outt",
                                             bufs=3)
                            nc.vector.tensor_copy(outt, po)
                            nc.sync.dma_start(
                                out=out[qtw * QT + qcw * 128:
                                        qtw * QT + (qcw + 1) * 128, :],
                                in_=outt)
                        if gi == 2 and pending_norm is not None:
                            # deferred normalize-multiply of the previous
                            # unit (its pv slot frees here, mid-unit, so the
                            # boundary never serializes on the norm chain)
                            emit_norm2(*pending_norm)
                            pending_norm = None
                        elif gi == 6 and pending_wo:
                            qtw, qcw = pending_wo.pop(0)
                            po = ppv.tile([128, D], F32, tag="pvb",
                                          name=f"po{qtw}_{qcw}")
                            for hh in range(4):
                                nc.tensor.matmul(
                                    po,
                                    xts[qtw][:, hh, qcw * 128:(qcw + 1) * 128],
                                    WO64[:, hh, :],
                                    start=(hh == 0), stop=False)
                            wo_inflight = (qtw, qcw, po)
                        if gi == len(groups) - 1:
                            if ui == len(units) - 1:
                                # final unit: fast-path norm via PE broadcast
                                # (score PSUM slots are free at this point)
                                rc = pwrk.tile([128, QT], F32, tag="rc",
                                               name="rcF")
                                nc.vector.reciprocal(rc[64:65, :],
                                                     pv[64:65, :])
                                bct = psc.tile([128, G, QT], F32, tag="sc",
                                               name="bcF")
                                nc.tensor.matmul(bct[0:64, 0, :],
                                                 ones_t[64:65, :],
                                                 rc[64:65, :],
                                                 start=True, stop=True)
                                bcs = pwrk.tile([64, QT], F32, tag="bcs",
                                                name="bcsF")
                                nc.vector.tensor_copy(bcs, bct[0:64, 0, :])
                                emit_norm2(qt, h, pv, bcs)
                                pending_wo.extend(
                                    (qt, qc) for qc in range(QT // 128))
                                while pending_wo:
                                    emit_wo(*pending_wo.pop(0))
                            else:
                                bcs = emit_norm1(ui, pv)
                                pending_norm = (qt, h, pv, bcs)
                                if h == H - 1:
                                    pending_wo.extend(
                                        (qt, qc) for qc in range(QT // 128))

    nc.compile()
    nc.m = get_hw_module(nc.m)
    return nc


def _get_built():
    global _BUILT
    if _BUILT is None:
        _BUILT = _build()
    return _BUILT


def kernel(q, k, v, mask, w_q, w_k, w_v, w_o):
    import os
    # NTFF tracing needs antenv.axon_hooks, absent in some environments;
    # never let an inherited BASS_TRACE env var route us into that path.
    os.environ.setdefault("BASS_NEVER_TRACE", "1")
    from concourse.bass_utils import run_bass_kernel_spmd

    q = np.asarray(q, dtype=np.float32)
    k = np.asarray(k, dtype=np.float32)
    v = np.asarray(v, dtype=np.float32)
    mask = np.asarray(mask, dtype=np.int32)
    w_q = np.asarray(w_q, dtype=np.float32)
    w_k = np.asarray(w_k, dtype=np.float32)
    w_v = np.asarray(w_v, dtype=np.float32)
    w_o = np.asarray(w_o, dtype=np.float32)

    nc = _get_built()

    kT = [np.ascontiguousarray(k[b].T) for b in range(B)]
    vT = [np.ascontiguousarray(v[b].T) for b in range(B)]
    maskT = [np.ascontiguousarray(mask[b].T) for b in range(B)]

    in_maps = []
    for c in range(NCORES):
        b, r = divmod(c, RG)
        q0 = r * QR
        in_maps.append({
            "qT": np.ascontiguousarray(q[b, q0:q0 + QR, :].T),
            "kT": kT[b],
            "vT": vT[b],
            "maskT": np.ascontiguousarray(maskT[b][:, q0:q0 + QR]),
            "wq": w_q, "wk": w_k, "wv": w_v, "wo": w_o,
        })

    global _LAST_IN_MAPS
    _LAST_IN_MAPS = in_maps
    res = run_bass_kernel_spmd(nc, in_maps, list(range(NCORES)))

    full = np.empty((B, S, D), dtype=np.float32)
    for c in range(NCORES):
        b, r = divmod(c, RG)
        full[b, r * QR:(r + 1) * QR, :] = res.results[c]["out"]
    return full

